# revision 1
# baseline (speedup 1.0000x reference)
"""KoLeo loss kernel for Trainium2, 8 NeuronCores (SPMD).

reference math:
    x = thought_vectors.reshape(-1, D)          # [N, D], N=8192, D=1024
    xn = x / max(||x||, 1e-12)
    sim = min(xn @ xn.T, 1.0)
    dist = sqrt(2 - 2*sim + 1e-4), diag -> inf
    loss = -mean(log(min_row_dist + 1e-8))

Key reduction: dist is monotone-decreasing in sim, so
    min_dist_i = sqrt(2 - 2*min(max_{j!=i} sim_ij, 1) + 1e-4)
and we only need a row-max of the Gram matrix (diag excluded).

Sharding: rows of x across 8 cores (1024 rows each). Each core
normalizes its shard, transposes it to [D, n] bf16, AllGathers the
transposed shards (2MB/rank), then computes its [1024, 8192] block of
the Gram matrix in 512-column chunks on the TensorEngine, keeping a
running row-max. Diagonal exclusion is done with a per-core additive
mask input (-30 on the diagonal entries). Final sqrt/log on-chip;
host sums the 8x[128x8] outputs into the scalar loss.
"""

import numpy as np

_P = 128
_NEG = -30.0
_EPS_NORM = 1e-12
_EPS_DIST = 1e-4
_EPSILON = 1e-8


def _build_program(ncores, NB, D, CHUNK):
    """Build the SPMD Bass program (one program, runs on all cores).

    NB: rows per core. D: feature dim. CHUNK: gram columns per chunk.
    """
    import concourse.bacc as bacc
    import concourse.mybir as mybir
    from concourse.tile import TileContext
    from concourse.masks import make_identity

    f32 = mybir.dt.float32
    bf16 = mybir.dt.bfloat16
    P = _P
    M_TILES = NB // P
    K_TILES = D // P
    N = NB * ncores
    NCHUNK = N // CHUNK
    CP = CHUNK // P  # m-tiles per chunk width
    assert NB % CHUNK == 0 and CHUNK % P == 0

    nc = bacc.Bacc(
        "TRN2", target_bir_lowering=False, debug=False, num_devices=ncores
    )
    xs = nc.dram_tensor("xs", [NB, D], f32, kind="ExternalInput")
    masks_in = nc.dram_tensor(
        "masks", [NCHUNK, P, M_TILES * P], bf16, kind="ExternalInput"
    )
    out = nc.dram_tensor("out", [P, M_TILES], f32, kind="ExternalOutput")

    with TileContext(nc) as tc:
        with (
            tc.tile_pool(name="consts", bufs=1) as consts,
            tc.tile_pool(name="dram", bufs=1, space="DRAM") as dram,
            tc.tile_pool(name="small", bufs=4) as small,
        ):
            identity = consts.tile([P, P], bf16)
            make_identity(nc, identity)
            bias_dist = consts.tile([P, 1], f32)
            nc.vector.memset(bias_dist, 2.0 + _EPS_DIST)
            bias_log = consts.tile([P, 1], f32)
            nc.vector.memset(bias_log, _EPSILON)
            # resident transposed normalized shard, split in halves so the
            # first AllGather's staging only depends on the first half
            MH = M_TILES // 2
            xnT_A = consts.tile([P, K_TILES, MH * P], bf16)
            xnT_B = consts.tile([P, K_TILES, MH * P], bf16)
            maxacc = consts.tile([P, M_TILES, NCHUNK], f32)
            outt = consts.tile([P, M_TILES], f32)

            HB = NB // 2
            assert CHUNK <= HB and HB % CHUNK == 0
            xnT_localA = dram.tile([D, HB], bf16)
            xnT_localB = dram.tile([D, HB], bf16)
            xnT_allA = dram.tile([ncores * D, HB], bf16, addr_space="Shared")
            xnT_allB = dram.tile([ncores * D, HB], bf16, addr_space="Shared")

            # ---- pre-pass: normalize own shard, transpose, stage for AG ----
            with (
                tc.tile_pool(name="prep", bufs=3) as prep,
                tc.tile_pool(name="ppsum", bufs=4, space="PSUM") as ppsum,
            ):
                for m in range(M_TILES):
                    xt = prep.tile([P, D], f32, tag="xt")
                    nc.sync.dma_start(xt, xs[m * P : (m + 1) * P, :])
                    sq = prep.tile([P, D], bf16, tag="sq")
                    ss = small.tile([P, 1], f32, tag="ss")
                    nc.scalar.activation(
                        sq,
                        xt,
                        mybir.ActivationFunctionType.Square,
                        accum_out=ss,
                    )
                    nrm = small.tile([P, 1], f32, tag="nrm")
                    nc.scalar.sqrt(nrm, ss)
                    nrm2 = small.tile([P, 1], f32, tag="nrm2")
                    nc.vector.tensor_scalar_max(nrm2, nrm, _EPS_NORM)
                    rinv = small.tile([P, 1], f32, tag="rinv")
                    nc.vector.reciprocal(rinv, nrm2)
                    xnb = prep.tile([P, D], bf16, tag="xnb")
                    nc.vector.tensor_scalar_mul(xnb, xt, rinv)
                    for k in range(K_TILES):
                        pt = ppsum.tile([P, P], bf16, tag="pt")
                        nc.tensor.transpose(
                            pt, xnb[:, k * P : (k + 1) * P], identity
                        )
                        xnT_h = xnT_A if m < MH else xnT_B
                        nc.vector.tensor_copy(
                            xnT_h[:, k, (m % MH) * P : (m % MH + 1) * P], pt
                        )
                    # stage + gather each half as soon as it is complete, so
                    # the first AllGather starts while the second half of the
                    # pre-pass is still running.
                    if m == M_TILES // 2 - 1:
                        for k in range(K_TILES):
                            nc.sync.dma_start(
                                xnT_localA[k * P : (k + 1) * P, :],
                                xnT_A[:, k, :],
                            )
                        nc.gpsimd.collective_compute(
                            "AllGather",
                            mybir.AluOpType.bypass,
                            replica_groups=[list(range(ncores))],
                            ins=[xnT_localA.opt()],
                            outs=[xnT_allA.opt()],
                        )
                    elif m == M_TILES - 1:
                        for k in range(K_TILES):
                            nc.sync.dma_start(
                                xnT_localB[k * P : (k + 1) * P, :],
                                xnT_B[:, k, :],
                            )
                        nc.gpsimd.collective_compute(
                            "AllGather",
                            mybir.AluOpType.bypass,
                            replica_groups=[list(range(ncores))],
                            ins=[xnT_localB.opt()],
                            outs=[xnT_allB.opt()],
                        )

            # ---- main pass: gram row-block in CHUNK-wide column chunks ----
            with (
                tc.tile_pool(name="rhsp", bufs=3 * K_TILES) as rhsp,
                tc.tile_pool(name="maskp", bufs=2) as maskp,
                tc.tile_pool(name="mpsum", bufs=8, space="PSUM") as mpsum,
            ):
                # process all first-half chunks, then second-half chunks:
                # compute on half A overlaps the AllGather of half B.
                j_order = [
                    j
                    for h in (0, 1)
                    for j in range(NCHUNK)
                    if ((j * CHUNK) % NB) // HB == h
                ]
                for j in j_order:
                    blk = (j * CHUNK) // NB  # source rank block
                    cib = (j * CHUNK) % NB  # col within block
                    half = cib // HB
                    nl0 = cib % HB
                    src = xnT_allA if half == 0 else xnT_allB
                    mask_t = maskp.tile([P, M_TILES * P], bf16, tag="mask")
                    nc.sync.dma_start(mask_t, masks_in[j])
                    rts = []
                    for k in range(K_TILES):
                        rt = rhsp.tile([P, CHUNK], bf16, tag="rhs")
                        nc.sync.dma_start(
                            rt,
                            src[
                                blk * D + k * P : blk * D + (k + 1) * P,
                                nl0 : nl0 + CHUNK,
                            ],
                        )
                        rts.append(rt)
                    for m in range(M_TILES):
                        ps = mpsum.tile([P, CHUNK], f32, tag="ps")
                        for k in range(K_TILES):
                            nc.tensor.matmul(
                                ps,
                                (xnT_A if m < MH else xnT_B)[
                                    :, k, (m % MH) * P : (m % MH + 1) * P
                                ],
                                rts[k],
                                start=(k == 0),
                                stop=(k == K_TILES - 1),
                            )
                        off = (m % CP) * P
                        nc.vector.tensor_add(
                            ps[:, off : off + P],
                            ps[:, off : off + P],
                            mask_t[:, m * P : (m + 1) * P],
                        )
                        nc.vector.reduce_max(
                            maxacc[:, m, j : j + 1],
                            ps,
                            axis=mybir.AxisListType.X,
                        )

            # ---- final: clamp, dist, log ----
            for m in range(M_TILES):
                mx = small.tile([P, 1], f32, tag="mx")
                nc.vector.reduce_max(
                    mx, maxacc[:, m, :], axis=mybir.AxisListType.X
                )
                mxc = small.tile([P, 1], f32, tag="mxc")
                nc.vector.tensor_scalar_min(mxc, mx, 1.0)
                dst = small.tile([P, 1], f32, tag="dst")
                nc.scalar.activation(
                    dst,
                    mxc,
                    mybir.ActivationFunctionType.Sqrt,
                    bias=bias_dist,
                    scale=-2.0,
                )
                nc.scalar.activation(
                    outt[:, m : m + 1],
                    dst,
                    mybir.ActivationFunctionType.Ln,
                    bias=bias_log,
                    scale=1.0,
                )
            nc.sync.dma_start(out[:, :], outt)

    nc.compile()
    return nc


def _make_masks(ncores, NB, D, CHUNK, core, np_bf16):
    """Per-core diag-exclusion masks: masks[j][p, m*P+e] = NEG iff chunk j
    holds m-tile m's diagonal block and e == p."""
    P = _P
    M_TILES = NB // P
    NCHUNK = (NB * ncores) // CHUNK
    masks = np.zeros((NCHUNK, P, M_TILES * P), dtype=np.float32)
    for m in range(M_TILES):
        g0 = core * NB + m * P  # global col of this m-tile's diagonal
        j = g0 // CHUNK
        for p in range(P):
            masks[j, p, m * P + p] = _NEG
    return masks.astype(np_bf16)


def _run(thought_vectors, trace=False, tmpdir=None):
    from concourse import mybir
    from concourse.bass_utils import run_bass_kernel_spmd

    np_bf16 = mybir.dt.np(mybir.dt.bfloat16)

    ncores, NB, D, CHUNK = 8, 1024, 1024, 512
    x = np.ascontiguousarray(
        np.asarray(thought_vectors, dtype=np.float32).reshape(-1, D)
    )
    N = x.shape[0]
    assert N == ncores * NB

    nc = _build_program(ncores, NB, D, CHUNK)

    in_maps = []
    for c in range(ncores):
        in_maps.append(
            {
                "xs": x[c * NB : (c + 1) * NB],
                "masks": _make_masks(ncores, NB, D, CHUNK, c, np_bf16),
            }
        )

    res = run_bass_kernel_spmd(
        nc,
        in_maps,
        core_ids=list(range(ncores)),
        trace=trace,
        tmpdir=tmpdir,
    )

    total = 0.0
    for c in range(ncores):
        total += float(np.asarray(res.results[c]["out"], dtype=np.float64).sum())
    loss = -total / N
    return np.float32(loss), res


def kernel(thought_vectors):
    loss, _ = _run(thought_vectors)
    return np.asarray(loss, dtype=np.float32)



# revision 3
# speedup vs baseline: 1.8136x; 1.8136x over previous
"""KoLeo loss kernel for Trainium2, 8 NeuronCores (SPMD), fp8 gram.

reference math:
    x = thought_vectors.reshape(-1, D)          # [N, D], N=8192, D=1024
    xn = x / max(||x||, 1e-12)
    sim = min(xn @ xn.T, 1.0)
    dist = sqrt(2 - 2*sim + 1e-4), diag -> inf
    loss = -mean(log(min_row_dist + 1e-8))

Key reduction: dist is monotone-decreasing in sim, so
    min_dist_i = sqrt(2 - 2*min(max_{j!=i} sim_ij, 1) + 1e-4)
and we only need a row-max of the Gram matrix (diag excluded).

Sharding: rows of x across 8 cores (1024 rows each). Each core
normalizes its shard scaled by 16, transposes it to [D, n] and stores
fp8e4 (values ~N(0, 0.25); quantization error ~2e-3 on the loss, far
inside the 2e-2 gate). The transposed fp8 shards are AllGathered
(1MB/rank), and each core computes its [1024, 8192] block of the
256x-scaled Gram with DoubleRow fp8 matmuls (2x TensorE throughput),
keeping a running row-max. Diagonal exclusion via per-core additive
mask input (-7680 on diag entries; only chunks with j = m//4 (mod 2)
can hold a diagonal block, so the mask covers [P, CHUNK] per chunk).
Final clamp/sqrt/log on-chip; host sums the 8x[128x8] outputs.
"""

import numpy as np

_P = 128
_NEG = -7680.0  # -30 * SCALE^2, exact in bf16
_EPS_NORM = 1e-12
_EPS_DIST = 1e-4
_EPSILON = 1e-8
_SCALE = 16.0  # fp8 pre-scale; gram is _SCALE**2 * sim


def _build_program(ncores, NB, D, CHUNK):
    """Build the SPMD Bass program (one program, runs on all cores).

    NB: rows per core. D: feature dim. CHUNK: gram columns per chunk.
    """
    import concourse.bacc as bacc
    import concourse.mybir as mybir
    from concourse.tile import TileContext
    from concourse.masks import make_identity

    f32 = mybir.dt.float32
    bf16 = mybir.dt.bfloat16
    fp8 = mybir.dt.float8e4
    P = _P
    M_TILES = NB // P
    K_TILES = D // P
    K2 = K_TILES // 2  # DoubleRow k-pair count
    N = NB * ncores
    NCHUNK = N // CHUNK
    CP = CHUNK // P  # m-tiles per chunk width
    S2 = _SCALE * _SCALE
    assert NB % CHUNK == 0 and CHUNK % P == 0

    nc = bacc.Bacc(
        "TRN2", target_bir_lowering=False, debug=False, num_devices=ncores
    )
    xs = nc.dram_tensor("xs", [NB, D], f32, kind="ExternalInput")
    masks_in = nc.dram_tensor(
        "masks", [NCHUNK, P, CHUNK], bf16, kind="ExternalInput"
    )
    out = nc.dram_tensor("out", [P, M_TILES], f32, kind="ExternalOutput")

    with TileContext(nc) as tc:
        with (
            tc.tile_pool(name="consts", bufs=1) as consts,
            tc.tile_pool(name="dram", bufs=1, space="DRAM") as dram,
            tc.tile_pool(name="small", bufs=4) as small,
        ):
            identity = consts.tile([P, P], bf16)
            make_identity(nc, identity)
            bias_dist = consts.tile([P, 1], f32)
            nc.vector.memset(bias_dist, 2.0 + _EPS_DIST)
            bias_log = consts.tile([P, 1], f32)
            nc.vector.memset(bias_log, _EPSILON)
            # resident transposed normalized shard (fp8, x16), split in
            # halves so the first AllGather only depends on the first half
            MH = M_TILES // 2
            xnT_A = consts.tile([P, K_TILES, MH * P], fp8)
            xnT_B = consts.tile([P, K_TILES, MH * P], fp8)
            maxacc = consts.tile([P, M_TILES, NCHUNK], f32)
            outt = consts.tile([P, M_TILES], f32)

            HB = NB // 2
            assert CHUNK <= HB and HB % CHUNK == 0
            xnT_localA = dram.tile([D, HB], fp8)
            xnT_localB = dram.tile([D, HB], fp8)
            xnT_allA = dram.tile([ncores * D, HB], fp8, addr_space="Shared")
            xnT_allB = dram.tile([ncores * D, HB], fp8, addr_space="Shared")

            # ---- pre-pass: normalize own shard, transpose, stage for AG ----
            with (
                tc.tile_pool(name="prep", bufs=3) as prep,
                tc.tile_pool(name="ppsum", bufs=4, space="PSUM") as ppsum,
            ):
                for m in range(M_TILES):
                    xt = prep.tile([P, D], f32, tag="xt")
                    nc.sync.dma_start(xt, xs[m * P : (m + 1) * P, :])
                    sq = prep.tile([P, D], bf16, tag="sq")
                    ss = small.tile([P, 1], f32, tag="ss")
                    nc.scalar.activation(
                        sq,
                        xt,
                        mybir.ActivationFunctionType.Square,
                        accum_out=ss,
                    )
                    nrm = small.tile([P, 1], f32, tag="nrm")
                    nc.scalar.sqrt(nrm, ss)
                    nrm2 = small.tile([P, 1], f32, tag="nrm2")
                    nc.vector.tensor_scalar_max(nrm2, nrm, _EPS_NORM)
                    rinv = small.tile([P, 1], f32, tag="rinv")
                    nc.vector.reciprocal(rinv, nrm2)
                    rinv16 = small.tile([P, 1], f32, tag="rinv16")
                    nc.vector.tensor_scalar_mul(rinv16, rinv, _SCALE)
                    xnb = prep.tile([P, D], bf16, tag="xnb")
                    nc.vector.tensor_scalar_mul(xnb, xt, rinv16)
                    for k in range(K_TILES):
                        pt = ppsum.tile([P, P], bf16, tag="pt")
                        nc.tensor.transpose(
                            pt, xnb[:, k * P : (k + 1) * P], identity
                        )
                        xnT_h = xnT_A if m < MH else xnT_B
                        nc.vector.tensor_copy(
                            xnT_h[:, k, (m % MH) * P : (m % MH + 1) * P], pt
                        )
                    # stage + gather each half as soon as it is complete, so
                    # the first AllGather starts while the second half of the
                    # pre-pass is still running.
                    if m == M_TILES // 2 - 1:
                        for k in range(K_TILES):
                            nc.sync.dma_start(
                                xnT_localA[k * P : (k + 1) * P, :],
                                xnT_A[:, k, :],
                            )
                        nc.gpsimd.collective_compute(
                            "AllGather",
                            mybir.AluOpType.bypass,
                            replica_groups=[list(range(ncores))],
                            ins=[xnT_localA.opt()],
                            outs=[xnT_allA.opt()],
                        )
                    elif m == M_TILES - 1:
                        for k in range(K_TILES):
                            nc.sync.dma_start(
                                xnT_localB[k * P : (k + 1) * P, :],
                                xnT_B[:, k, :],
                            )
                        nc.gpsimd.collective_compute(
                            "AllGather",
                            mybir.AluOpType.bypass,
                            replica_groups=[list(range(ncores))],
                            ins=[xnT_localB.opt()],
                            outs=[xnT_allB.opt()],
                        )

            # ---- main pass: gram row-block in CHUNK-wide column chunks ----
            with (
                tc.tile_pool(name="rhsp", bufs=3 * K2) as rhsp,
                tc.tile_pool(name="maskp", bufs=2) as maskp,
                tc.tile_pool(name="mpsum", bufs=8, space="PSUM") as mpsum,
            ):
                # process all first-half chunks, then second-half chunks:
                # compute on half A overlaps the AllGather of half B.
                j_order = [
                    j
                    for h in (0, 1)
                    for j in range(NCHUNK)
                    if ((j * CHUNK) % NB) // HB == h
                ]
                for j in j_order:
                    blk = (j * CHUNK) // NB  # source rank block
                    cib = (j * CHUNK) % NB  # col within block
                    half = cib // HB
                    nl0 = cib % HB
                    src = xnT_allA if half == 0 else xnT_allB
                    mask_t = maskp.tile([P, CHUNK], bf16, tag="mask")
                    nc.sync.dma_start(mask_t, masks_in[j])
                    rts = []
                    for k2 in range(K2):
                        rt = rhsp.tile([P, 2, CHUNK], fp8, tag="rhs")
                        for t in range(2):
                            k = 2 * k2 + t
                            nc.sync.dma_start(
                                rt[:, t, :],
                                src[
                                    blk * D + k * P : blk * D + (k + 1) * P,
                                    nl0 : nl0 + CHUNK,
                                ],
                            )
                        rts.append(rt)
                    for m in range(M_TILES):
                        ps = mpsum.tile([P, CHUNK], f32, tag="ps")
                        lhs_h = xnT_A if m < MH else xnT_B
                        mc = (m % MH) * P
                        for k2 in range(K2):
                            nc.tensor.matmul(
                                ps,
                                lhs_h[:, 2 * k2 : 2 * k2 + 2, mc : mc + P],
                                rts[k2],
                                start=(k2 == 0),
                                stop=(k2 == K2 - 1),
                                perf_mode=mybir.MatmulPerfMode.DoubleRow,
                            )
                        # the diagonal block of m-tile m lands in a chunk j
                        # with j = 2*rank + m//CP, so only m//CP = j%2 pairs
                        # can carry it (for any rank); mask those only.
                        if m // CP == j % 2:
                            off = (m % CP) * P
                            nc.vector.tensor_add(
                                ps[:, off : off + P],
                                ps[:, off : off + P],
                                mask_t[:, off : off + P],
                            )
                        nc.vector.reduce_max(
                            maxacc[:, m, j : j + 1],
                            ps,
                            axis=mybir.AxisListType.X,
                        )

            # ---- final: clamp, dist, log ----
            for m in range(M_TILES):
                mx = small.tile([P, 1], f32, tag="mx")
                nc.vector.reduce_max(
                    mx, maxacc[:, m, :], axis=mybir.AxisListType.X
                )
                mxc = small.tile([P, 1], f32, tag="mxc")
                nc.vector.tensor_scalar_min(mxc, mx, S2)
                dst = small.tile([P, 1], f32, tag="dst")
                nc.scalar.activation(
                    dst,
                    mxc,
                    mybir.ActivationFunctionType.Sqrt,
                    bias=bias_dist,
                    scale=-2.0 / S2,
                )
                nc.scalar.activation(
                    outt[:, m : m + 1],
                    dst,
                    mybir.ActivationFunctionType.Ln,
                    bias=bias_log,
                    scale=1.0,
                )
            nc.sync.dma_start(out[:, :], outt)

    nc.compile()
    return nc


def _make_masks(ncores, NB, D, CHUNK, core, np_bf16):
    """Per-core diag-exclusion masks: masks[j][p, (m%CP)*P+p] = NEG iff
    chunk j holds m-tile m's diagonal block (m = row p's tile)."""
    P = _P
    M_TILES = NB // P
    NCHUNK = (NB * ncores) // CHUNK
    CP = CHUNK // P
    masks = np.zeros((NCHUNK, P, CHUNK), dtype=np.float32)
    for m in range(M_TILES):
        g0 = core * NB + m * P  # global col of this m-tile's diagonal
        j = g0 // CHUNK
        off = (g0 % CHUNK) // P
        assert off == m % CP and m // CP == j % 2
        for p in range(P):
            masks[j, p, off * P + p] = _NEG
    return masks.astype(np_bf16)


def _run(thought_vectors, trace=False, tmpdir=None):
    from concourse import mybir
    from concourse.bass_utils import run_bass_kernel_spmd

    np_bf16 = mybir.dt.np(mybir.dt.bfloat16)

    ncores, NB, D, CHUNK = 8, 1024, 1024, 512
    x = np.ascontiguousarray(
        np.asarray(thought_vectors, dtype=np.float32).reshape(-1, D)
    )
    N = x.shape[0]
    assert N == ncores * NB

    nc = _build_program(ncores, NB, D, CHUNK)

    in_maps = []
    for c in range(ncores):
        in_maps.append(
            {
                "xs": x[c * NB : (c + 1) * NB],
                "masks": _make_masks(ncores, NB, D, CHUNK, c, np_bf16),
            }
        )

    res = run_bass_kernel_spmd(
        nc,
        in_maps,
        core_ids=list(range(ncores)),
        trace=trace,
        tmpdir=tmpdir,
    )

    total = 0.0
    for c in range(ncores):
        total += float(np.asarray(res.results[c]["out"], dtype=np.float64).sum())
    loss = -total / N
    return np.float32(loss), res


def kernel(thought_vectors):
    loss, _ = _run(thought_vectors)
    return np.asarray(loss, dtype=np.float32)
